# revision 2
# baseline (speedup 1.0000x reference)
"""HadamardTrustQuantizer Trainium2 kernel, v4 (host block-transposed input).

Forward math (mask term cancels):
    y   = blockwise_rot(x, H)          # H: 128x128 Hadamard
    std = max(sqrt(mean(y^2, -1)), 1e-8) = max(sqrt(mean(x^2, -1)), 1e-8)
    step = ALPHA*std/QMAX
    q   = clip(round(y/step), -QMAX, QMAX)
    out = blockwise_rot(q*step, H)

v4 strategy (per core, data-parallel shard of 2048 rows):
  - host hands x BLOCK-TRANSPOSED: x_fm[b, f, r] (pure layout change), fp16.
    This kills the on-device x-transposes and their PSUM drains.
  - row sum-of-squares: square chunks on DVE (2x), reduce over features with
    PE ones-matmuls accumulating into a [1, rows] PSUM strip; a K=1 fp32
    matmul transposes the strip to [rows, 1] for per-partition use.
  - mm1 row-major: lhsT = x-strip (stationary), rhs = Hq = sign(H)/8 moving;
    output y'' lands [row, feat] so the ACT drain fuses prescale + round:
    q' = fp16(y''*rs + 1536) = 1536 + round_even(u), rs per-partition.
  - un-offset + top clip fused into the q-transpose drain (DVE 2x):
    (q' - 1536) then min 7; bottom clip (max -7) one-op pass on GPSIMD.
  - mm2: stationary q (feature-major), moving hs = +-1 sign matrix; output
    row-major; ACT/DVE drain applies os = step/sqrt(128) per-partition.
"""

import math
import sys

sys.path.insert(0, "/opt/trn_rl_repo")

import ml_dtypes  # noqa: F401
import numpy as np

import concourse.bass as bass
import concourse.tile as tile
from concourse import mybir
from concourse.bass_utils import run_bass_kernel_spmd

P = 128
NCOLS = 4096
NB = NCOLS // P          # 32 blocks per row
ALPHA = 2.5139
QMAX = 7.0
OFF = 1536.0             # fp16 round-to-integer offset (ulp=1 in [1024,2048))
# s1 = sqrt(ssq * CS) = step*sqrt(2), where step = ALPHA*sqrt(ssq/4096)/QMAX
CS = (ALPHA * math.sqrt(2.0) / (QMAX * 64.0)) ** 2
# os = step/sqrt(128) = s1/16

N_CORES = 8
ROWS_PER_CORE = 2048
RCHUNK = 256             # rows per DMA chunk

F32 = mybir.dt.float32
F16 = mybir.dt.float16
Alu = mybir.AluOpType
Act = mybir.ActivationFunctionType


def _split_waits(nc, maxw_default=1, drain_maxw=1):
    """walrus in this container rejects >1 sem wait per instruction.
    Hoist excess waits onto preceding same-engine NoOps."""
    for bb in nc.m.functions[0].blocks:
        new_list, changed = [], False
        for inst in bb.instructions:
            si = inst.sync_info
            maxw = drain_maxw if type(inst).__name__ == "InstDrain" else maxw_default
            if si is not None and len(si.on_wait) > maxw:
                waits = list(si.on_wait)
                head, tail = waits[:-maxw], waits[-maxw:]
                k = 0
                while head:
                    chunk, head = head[:1], head[1:]
                    nop = mybir.InstNoOp(name=f"{inst.name}-ws{k}", ins=[], outs=[])
                    nop.engine = inst.engine
                    nop.sync_info = mybir.SyncInfo(on_wait=chunk, on_update=[])
                    new_list.append(nop)
                    k += 1
                inst.sync_info = mybir.SyncInfo(
                    on_wait=tail, on_update=list(si.on_update)
                )
                changed = True
            new_list.append(inst)
        if changed:
            bb.instructions = new_list


def build(nrows=ROWS_PER_CORE, split_waits=True,
          act_outdrains=(0, 1, 0, 1, 0, 1, 0, 1), braid=True):
    """Per-core program; input x_fm[NB, 128, nrows] fp16 (block-transposed)."""
    assert nrows % RCHUNK == 0
    n_chunks = nrows // RCHUNK
    tiles_per_chunk = RCHUNK // P
    n_tiles = nrows // P

    nc = bass.Bass("TRN2", target_bir_lowering=False)
    x_d = nc.dram_tensor("x", [NB, P, nrows], F16, kind="ExternalInput")
    hq_d = nc.dram_tensor("hq", [P, P], F16, kind="ExternalInput")
    hs_d = nc.dram_tensor("hs", [P, P], F16, kind="ExternalInput")
    id_d = nc.dram_tensor("ident", [P, P], F16, kind="ExternalInput")
    ones_d = nc.dram_tensor("onesc", [P, 1], F16, kind="ExternalInput")
    one1_d = nc.dram_tensor("one1", [1, 1], F32, kind="ExternalInput")
    o_d = nc.dram_tensor("o", [nrows, NCOLS], F16, kind="ExternalOutput")

    with tile.TileContext(nc) as tc:
        import contextlib

        with contextlib.ExitStack() as ctx:
            singles = ctx.enter_context(tc.tile_pool(name="singles", bufs=1))
            px = ctx.enter_context(tc.tile_pool(name="px", bufs=3))
            psq = ctx.enter_context(tc.tile_pool(name="psq", bufs=2))
            pq = ctx.enter_context(tc.tile_pool(name="pq", bufs=3))
            pqT = ctx.enter_context(tc.tile_pool(name="pqT", bufs=6))
            pqc = ctx.enter_context(tc.tile_pool(name="pqc", bufs=6))
            pout = ctx.enter_context(tc.tile_pool(name="pout", bufs=3))
            pst = ctx.enter_context(tc.tile_pool(name="pst", bufs=6))
            pss = ctx.enter_context(tc.tile_pool(name="pss", bufs=1, space="PSUM"))
            psT = ctx.enter_context(tc.tile_pool(name="psT", bufs=1, space="PSUM"))
            pyp = ctx.enter_context(tc.tile_pool(name="pyp", bufs=2, space="PSUM"))
            ptq = ctx.enter_context(tc.tile_pool(name="ptq", bufs=2, space="PSUM"))
            pop = ctx.enter_context(tc.tile_pool(name="pop", bufs=2, space="PSUM"))

            hq_sb = singles.tile([P, P], F16)
            hs_sb = singles.tile([P, P], F16)
            id_sb = singles.tile([P, P], F16)
            ones_sb = singles.tile([P, 1], F16)
            one1_sb = singles.tile([1, 1], F32)
            nc.sync.dma_start(out=hq_sb, in_=hq_d[:])
            nc.sync.dma_start(out=hs_sb, in_=hs_d[:])
            nc.sync.dma_start(out=id_sb, in_=id_d[:])
            nc.sync.dma_start(out=ones_sb, in_=ones_d[:])
            nc.sync.dma_start(out=one1_sb, in_=one1_d[:])

            x_chunks = {}

            def emit_dma(c):
                if c >= n_chunks:
                    return
                xs = px.tile([P, NB, RCHUNK], F16, tag="x")
                src = x_d[:, :, c * RCHUNK : (c + 1) * RCHUNK]
                nc.sync.dma_start(out=xs, in_=src.rearrange("b p r -> p b r"))
                x_chunks[c] = xs

            emit_dma(0)
            phase_bs = []

            for c in range(n_chunks):
                emit_dma(c + 1)
                xs = x_chunks.pop(c)

                # squares for the whole chunk (DVE 2x), in 4 wide slabs
                xsq = psq.tile([P, NB, RCHUNK], F16, tag="xsq")
                for s in range(4):
                    b0 = s * (NB // 4)
                    b1 = (s + 1) * (NB // 4)
                    nc.vector.tensor_tensor(
                        out=xsq[:, b0:b1, :], in0=xs[:, b0:b1, :],
                        in1=xs[:, b0:b1, :], op=Alu.mult,
                    )

                for t in range(tiles_per_chunk):
                    rt = c * tiles_per_chunk + t
                    r0 = rt * P
                    cs0 = t * P

                    # ssq[rows]: PE ones-matmuls accumulate into [1, 128]
                    sq_ps = pss.tile([1, P], F32, tag="sq")
                    for b in range(NB):
                        nc.tensor.matmul(
                            sq_ps, lhsT=ones_sb,
                            rhs=xsq[:, b, cs0 : cs0 + P],
                            start=(b == 0), stop=(b == NB - 1),
                        )
                    sq_sb = pst.tile([1, P], F32, tag="sqsb")
                    nc.vector.tensor_copy(out=sq_sb, in_=sq_ps)
                    sqT = psT.tile([P, 1], F32, tag="sqT")
                    nc.tensor.matmul(sqT, lhsT=sq_sb, rhs=one1_sb,
                                     start=True, stop=True)
                    # s1 = step*sqrt(2) = sqrt(ssq*CS); rs = 1/s1; os = s1/16
                    s1 = pst.tile([P, 1], F32, tag="s1")
                    nc.scalar.activation(out=s1, in_=sqT, func=Act.Sqrt, scale=CS)
                    rs = pst.tile([P, 1], F32, tag="rs")
                    nc.vector.reciprocal(out=rs, in_=s1)
                    os_t = pst.tile([P, 1], F32, tag="os")
                    nc.scalar.activation(out=os_t, in_=s1, func=Act.Copy,
                                         scale=1.0 / 16.0)

                    # mm1 row-major + fused ACT prescale/round/offset
                    q = pq.tile([P, NCOLS], F16, tag="q")
                    for g in range(8):
                        yp = pyp.tile([P, 512], F32, tag="yp")
                        for j in range(4):
                            b = 4 * g + j
                            nc.tensor.matmul(
                                yp[:, j * P : (j + 1) * P],
                                lhsT=xs[:, b, cs0 : cs0 + P],
                                rhs=hq_sb,
                                start=True, stop=True,
                            )
                        nc.scalar.activation(
                            out=q[:, g * 512 : (g + 1) * 512], in_=yp,
                            func=Act.Copy, scale=rs[:, 0:1], bias=OFF,
                        )

                    def phase_b(q=q, os_t=os_t, r0=r0):
                        out_t = pout.tile([P, NCOLS], F16, tag="out")
                        for g in range(4):
                            tq = ptq.tile([P, 1024], F16, tag="tq")
                            for k in range(8):
                                b = 8 * g + k
                                nc.tensor.transpose(
                                    tq[:, k * P : (k + 1) * P],
                                    q[:, b * P : (b + 1) * P],
                                    id_sb,
                                )
                            qT = pqT.tile([P, 1024], F16, tag="qT")
                            nc.vector.tensor_scalar(
                                out=qT, in0=tq, scalar1=OFF, scalar2=QMAX,
                                op0=Alu.subtract, op1=Alu.min,
                            )
                            qc = pqc.tile([P, 1024], F16, tag="qc")
                            nc.gpsimd.tensor_scalar(
                                out=qc, in0=qT, scalar1=-QMAX, scalar2=None,
                                op0=Alu.max,
                            )
                            for h in range(2):
                                bank = 2 * g + h
                                op_t = pop.tile([P, 512], F32, tag="op")
                                for j in range(4):
                                    k = h * 4 + j
                                    nc.tensor.matmul(
                                        op_t[:, j * P : (j + 1) * P],
                                        lhsT=qc[:, k * P : (k + 1) * P],
                                        rhs=hs_sb,
                                        start=True, stop=True,
                                    )
                                c0 = g * 1024 + h * 512
                                if act_outdrains[bank]:
                                    nc.scalar.activation(
                                        out=out_t[:, c0 : c0 + 512], in_=op_t,
                                        func=Act.Copy, scale=os_t[:, 0:1],
                                    )
                                else:
                                    nc.vector.tensor_scalar(
                                        out=out_t[:, c0 : c0 + 512], in0=op_t,
                                        scalar1=os_t[:, 0:1], scalar2=None,
                                        op0=Alu.mult,
                                    )
                            yield None
                        nc.sync.dma_start(out=o_d[r0 : r0 + P, :], in_=out_t)
                        yield None

                    if braid:
                        if phase_bs:
                            for _ in phase_bs.pop(0):
                                pass
                        phase_bs.append(phase_b())
                    else:
                        phase_bs.append(phase_b())
                        for _ in phase_bs.pop(0):
                            pass
            while phase_bs:
                for _ in phase_bs.pop(0):
                    pass

    if split_waits:
        _split_waits(nc)
    return nc


_NC_CACHE = {}


def _get_nc(nrows):
    if nrows not in _NC_CACHE:
        _NC_CACHE[nrows] = build(nrows)
    return _NC_CACHE[nrows]


def make_aux(H):
    H32 = np.asarray(H, dtype=np.float32)
    hq = (np.sign(H32) * 0.125).astype(np.float16)
    hs = np.sign(H32).astype(np.float16)
    ident = np.eye(P, dtype=np.float16)
    onesc = np.ones((P, 1), dtype=np.float16)
    one1 = np.ones((1, 1), dtype=np.float32)
    return hq, hs, ident, onesc, one1


def block_transpose(xf16):
    """[rows, 4096] fp16 -> [NB, 128, rows] fp16 (pure layout)."""
    r = xf16.shape[0]
    return np.ascontiguousarray(
        xf16.reshape(r, NB, P).transpose(1, 2, 0)
    )


def kernel(x, H):
    x = np.asarray(x)
    orig_shape = x.shape
    xf = x.reshape(-1, NCOLS).astype(np.float16)
    nrows_total = xf.shape[0]
    assert nrows_total % N_CORES == 0
    shard = nrows_total // N_CORES

    hq, hs, ident, onesc, one1 = make_aux(H)
    nc = _get_nc(shard)

    in_maps = [
        {
            "x": block_transpose(xf[i * shard : (i + 1) * shard]),
            "hq": hq,
            "hs": hs,
            "ident": ident,
            "onesc": onesc,
            "one1": one1,
        }
        for i in range(N_CORES)
    ]
    res = run_bass_kernel_spmd(nc, in_maps, core_ids=list(range(N_CORES)))
    out = np.concatenate([r["o"] for r in res.results], axis=0)
    return out.reshape(orig_shape).astype(np.float32)


if __name__ == "__main__":
    rng = np.random.default_rng(0)
    nrows = 512
    x32 = rng.standard_normal((nrows, NCOLS), dtype=np.float32)

    Hnp = np.ones((1, 1))
    while Hnp.shape[0] < P:
        Hnp = np.block([[Hnp, Hnp], [Hnp, -Hnp]])
    Hnp = (Hnp / math.sqrt(P)).astype(np.float32)

    def ref(x, H):
        xr = (x.reshape(-1, NB, P) @ H).reshape(-1, NCOLS)
        std = np.maximum(np.sqrt((xr * xr).mean(-1, keepdims=True)), 1e-8)
        step = ALPHA * std / QMAX
        q = np.clip(np.round(xr / step), -QMAX, QMAX) * step
        return (q.reshape(-1, NB, P) @ H).reshape(-1, NCOLS)

    from concourse.bass_interp import CoreSim
    from concourse.timeline_sim import TimelineSim

    nc = build(nrows, split_waits=False)
    hq, hs, ident, onesc, one1 = make_aux(Hnp)
    x16 = x32.astype(np.float16)
    sim = CoreSim(nc)
    sim.tensor("x")[:] = block_transpose(x16)
    sim.tensor("hq")[:] = hq
    sim.tensor("hs")[:] = hs
    sim.tensor("ident")[:] = ident
    sim.tensor("onesc")[:] = onesc
    sim.tensor("one1")[:] = one1
    sim.simulate()
    got = np.asarray(sim.tensor("o")).astype(np.float32)
    want = ref(x32, Hnp)
    err = np.abs(got - want)
    denom = np.abs(want).max()
    l2 = np.linalg.norm((got - want).ravel()) / np.linalg.norm(want.ravel())
    print("max abs err:", err.max(), "rel l2:", l2)
    bad = (err > 1e-3 * denom).sum()
    print("elements off by >1e-3*scale:", bad, "/", err.size)

    nc2 = build(nrows)
    ts = TimelineSim(nc2)
    ts.simulate()
    print(f"timeline {nrows} rows: {ts.time:.0f}ns")


# revision 3
# speedup vs baseline: 1.0300x; 1.0300x over previous
"""HadamardTrustQuantizer Trainium2 kernel, v4 (host block-transposed input).

Forward math (mask term cancels):
    y   = blockwise_rot(x, H)          # H: 128x128 Hadamard
    std = max(sqrt(mean(y^2, -1)), 1e-8) = max(sqrt(mean(x^2, -1)), 1e-8)
    step = ALPHA*std/QMAX
    q   = clip(round(y/step), -QMAX, QMAX)
    out = blockwise_rot(q*step, H)

v4 strategy (per core, data-parallel shard of 2048 rows):
  - host hands x BLOCK-TRANSPOSED: x_fm[b, f, r] (pure layout change), fp16.
    This kills the on-device x-transposes and their PSUM drains.
  - row sum-of-squares: square chunks on DVE (2x), reduce over features with
    PE ones-matmuls accumulating into a [1, rows] PSUM strip; a K=1 fp32
    matmul transposes the strip to [rows, 1] for per-partition use.
  - mm1 row-major: lhsT = x-strip (stationary), rhs = Hq = sign(H)/8 moving;
    output y'' lands [row, feat] so the ACT drain fuses prescale + round:
    q' = fp16(y''*rs + 1536) = 1536 + round_even(u), rs per-partition.
  - un-offset + top clip fused into the q-transpose drain (DVE 2x):
    (q' - 1536) then min 7; bottom clip (max -7) one-op pass on GPSIMD.
  - mm2: stationary q (feature-major), moving hs = +-1 sign matrix; output
    row-major; ACT/DVE drain applies os = step/sqrt(128) per-partition.
"""

import math
import sys

sys.path.insert(0, "/opt/trn_rl_repo")

import ml_dtypes  # noqa: F401
import numpy as np

import concourse.bass as bass
import concourse.tile as tile
from concourse import mybir
from concourse.bass_utils import run_bass_kernel_spmd

P = 128
NCOLS = 4096
NB = NCOLS // P          # 32 blocks per row
ALPHA = 2.5139
QMAX = 7.0
OFF = 1536.0             # fp16 round-to-integer offset (ulp=1 in [1024,2048))
# s1 = sqrt(ssq * CS) = step*sqrt(2), where step = ALPHA*sqrt(ssq/4096)/QMAX
CS = (ALPHA * math.sqrt(2.0) / (QMAX * 64.0)) ** 2
# os = step/sqrt(128) = s1/16

N_CORES = 8
ROWS_PER_CORE = 2048
RCHUNK = 256             # rows per DMA chunk

F32 = mybir.dt.float32
F16 = mybir.dt.float16
Alu = mybir.AluOpType
Act = mybir.ActivationFunctionType


def _split_waits(nc, maxw_default=1, drain_maxw=1):
    """walrus in this container rejects >1 sem wait per instruction.
    Hoist excess waits onto preceding same-engine NoOps."""
    for bb in nc.m.functions[0].blocks:
        new_list, changed = [], False
        for inst in bb.instructions:
            si = inst.sync_info
            maxw = drain_maxw if type(inst).__name__ == "InstDrain" else maxw_default
            if si is not None and len(si.on_wait) > maxw:
                waits = list(si.on_wait)
                head, tail = waits[:-maxw], waits[-maxw:]
                k = 0
                while head:
                    chunk, head = head[:1], head[1:]
                    nop = mybir.InstNoOp(name=f"{inst.name}-ws{k}", ins=[], outs=[])
                    nop.engine = inst.engine
                    nop.sync_info = mybir.SyncInfo(on_wait=chunk, on_update=[])
                    new_list.append(nop)
                    k += 1
                inst.sync_info = mybir.SyncInfo(
                    on_wait=tail, on_update=list(si.on_update)
                )
                changed = True
            new_list.append(inst)
        if changed:
            bb.instructions = new_list


def build(nrows=ROWS_PER_CORE, split_waits=True,
          act_outdrains=(0, 1, 0, 1, 0, 1, 0, 1), braid=True):
    """Per-core program; input x_fm[NB, 128, nrows] fp16 (block-transposed)."""
    assert nrows % RCHUNK == 0
    n_chunks = nrows // RCHUNK
    tiles_per_chunk = RCHUNK // P
    n_tiles = nrows // P

    nc = bass.Bass("TRN2", target_bir_lowering=False)
    x_d = nc.dram_tensor("x", [NB, P, nrows], F16, kind="ExternalInput")
    hq_d = nc.dram_tensor("hq", [P, P], F16, kind="ExternalInput")
    hs_d = nc.dram_tensor("hs", [P, P], F16, kind="ExternalInput")
    id_d = nc.dram_tensor("ident", [P, P], F16, kind="ExternalInput")
    ones_d = nc.dram_tensor("onesc", [P, 1], F16, kind="ExternalInput")
    one1_d = nc.dram_tensor("one1", [1, 1], F32, kind="ExternalInput")
    o_d = nc.dram_tensor("o", [nrows, NCOLS], F16, kind="ExternalOutput")

    with tile.TileContext(nc) as tc:
        import contextlib

        with contextlib.ExitStack() as ctx:
            singles = ctx.enter_context(tc.tile_pool(name="singles", bufs=1))
            px = ctx.enter_context(tc.tile_pool(name="px", bufs=3))
            psq = ctx.enter_context(tc.tile_pool(name="psq", bufs=2))
            pq = ctx.enter_context(tc.tile_pool(name="pq", bufs=3))
            pqT = ctx.enter_context(tc.tile_pool(name="pqT", bufs=6))
            pqc = ctx.enter_context(tc.tile_pool(name="pqc", bufs=6))
            pout = ctx.enter_context(tc.tile_pool(name="pout", bufs=3))
            pst = ctx.enter_context(tc.tile_pool(name="pst", bufs=6))
            pss = ctx.enter_context(tc.tile_pool(name="pss", bufs=1, space="PSUM"))
            psT = ctx.enter_context(tc.tile_pool(name="psT", bufs=1, space="PSUM"))
            pyp = ctx.enter_context(tc.tile_pool(name="pyp", bufs=2, space="PSUM"))
            ptq = ctx.enter_context(tc.tile_pool(name="ptq", bufs=2, space="PSUM"))
            pop = ctx.enter_context(tc.tile_pool(name="pop", bufs=2, space="PSUM"))

            hq_sb = singles.tile([P, P], F16)
            hs_sb = singles.tile([P, P], F16)
            id_sb = singles.tile([P, P], F16)
            ones_sb = singles.tile([P, 1], F16)
            one1_sb = singles.tile([1, 1], F32)
            nc.sync.dma_start(out=hq_sb, in_=hq_d[:])
            nc.sync.dma_start(out=hs_sb, in_=hs_d[:])
            nc.sync.dma_start(out=id_sb, in_=id_d[:])
            nc.sync.dma_start(out=ones_sb, in_=ones_d[:])
            nc.sync.dma_start(out=one1_sb, in_=one1_d[:])

            x_chunks = {}

            def emit_dma(c):
                if c >= n_chunks:
                    return
                xs = px.tile([P, NB, RCHUNK], F16, tag="x")
                src = x_d[:, :, c * RCHUNK : (c + 1) * RCHUNK]
                nc.sync.dma_start(out=xs, in_=src.rearrange("b p r -> p b r"))
                x_chunks[c] = xs

            emit_dma(0)
            phase_bs = []

            for c in range(n_chunks):
                emit_dma(c + 1)
                xs = x_chunks.pop(c)

                # squares for the whole chunk (DVE 2x), in 4 wide slabs
                xsq = psq.tile([P, NB, RCHUNK], F16, tag="xsq")
                for s in range(4):
                    b0 = s * (NB // 4)
                    b1 = (s + 1) * (NB // 4)
                    nc.vector.tensor_tensor(
                        out=xsq[:, b0:b1, :], in0=xs[:, b0:b1, :],
                        in1=xs[:, b0:b1, :], op=Alu.mult,
                    )

                # stats for all tiles of the chunk, hoisted so rs/os are
                # ready well before the ACT rounds need them
                stats = []
                for t in range(tiles_per_chunk):
                    cs0 = t * P
                    # ssq[rows]: PE ones-matmuls accumulate into [1, 128]
                    sq_ps = pss.tile([1, P], F32, tag="sq")
                    for b in range(NB):
                        nc.tensor.matmul(
                            sq_ps, lhsT=ones_sb,
                            rhs=xsq[:, b, cs0 : cs0 + P],
                            start=(b == 0), stop=(b == NB - 1),
                        )
                    sq_sb = pst.tile([1, P], F32, tag="sqsb")
                    nc.vector.tensor_copy(out=sq_sb, in_=sq_ps)
                    sqT = psT.tile([P, 1], F32, tag="sqT")
                    nc.tensor.matmul(sqT, lhsT=sq_sb, rhs=one1_sb,
                                     start=True, stop=True)
                    # rs = 1/(step*sqrt(2)) = sqrt(1/(ssq*CS));
                    # os = step/sqrt(128) = sqrt(ssq*CS)/16
                    rcp = pst.tile([P, 1], F32, tag="rcp")
                    nc.vector.reciprocal(out=rcp, in_=sqT)
                    rs = pst.tile([P, 1], F32, tag="rs")
                    nc.scalar.activation(out=rs, in_=rcp, func=Act.Sqrt,
                                         scale=1.0 / CS)
                    os_t = pst.tile([P, 1], F32, tag="os")
                    nc.scalar.activation(out=os_t, in_=sqT, func=Act.Sqrt,
                                         scale=CS / 256.0)
                    stats.append((rs, os_t))

                for t in range(tiles_per_chunk):
                    rt = c * tiles_per_chunk + t
                    r0 = rt * P
                    cs0 = t * P
                    rs, os_t = stats[t]

                    # mm1 row-major + fused ACT prescale/round/offset
                    q = pq.tile([P, NCOLS], F16, tag="q")
                    for g in range(8):
                        yp = pyp.tile([P, 512], F32, tag="yp")
                        for j in range(4):
                            b = 4 * g + j
                            nc.tensor.matmul(
                                yp[:, j * P : (j + 1) * P],
                                lhsT=xs[:, b, cs0 : cs0 + P],
                                rhs=hq_sb,
                                start=True, stop=True,
                            )
                        nc.scalar.activation(
                            out=q[:, g * 512 : (g + 1) * 512], in_=yp,
                            func=Act.Copy, scale=rs[:, 0:1], bias=OFF,
                        )

                    def phase_b(q=q, os_t=os_t, r0=r0):
                        out_t = pout.tile([P, NCOLS], F16, tag="out")
                        for g in range(4):
                            tq = ptq.tile([P, 1024], F16, tag="tq")
                            for k in range(8):
                                b = 8 * g + k
                                nc.tensor.transpose(
                                    tq[:, k * P : (k + 1) * P],
                                    q[:, b * P : (b + 1) * P],
                                    id_sb,
                                )
                            qT = pqT.tile([P, 1024], F16, tag="qT")
                            nc.vector.tensor_scalar(
                                out=qT, in0=tq, scalar1=OFF, scalar2=QMAX,
                                op0=Alu.subtract, op1=Alu.min,
                            )
                            qc = pqc.tile([P, 1024], F16, tag="qc")
                            nc.gpsimd.tensor_scalar(
                                out=qc, in0=qT, scalar1=-QMAX, scalar2=None,
                                op0=Alu.max,
                            )
                            for h in range(2):
                                bank = 2 * g + h
                                op_t = pop.tile([P, 512], F32, tag="op")
                                for j in range(4):
                                    k = h * 4 + j
                                    nc.tensor.matmul(
                                        op_t[:, j * P : (j + 1) * P],
                                        lhsT=qc[:, k * P : (k + 1) * P],
                                        rhs=hs_sb,
                                        start=True, stop=True,
                                    )
                                c0 = g * 1024 + h * 512
                                if act_outdrains[bank]:
                                    nc.scalar.activation(
                                        out=out_t[:, c0 : c0 + 512], in_=op_t,
                                        func=Act.Copy, scale=os_t[:, 0:1],
                                    )
                                else:
                                    nc.vector.tensor_scalar(
                                        out=out_t[:, c0 : c0 + 512], in0=op_t,
                                        scalar1=os_t[:, 0:1], scalar2=None,
                                        op0=Alu.mult,
                                    )
                            yield None
                        nc.sync.dma_start(out=o_d[r0 : r0 + P, 0:2048], in_=out_t[:, 0:2048])
                        nc.sync.dma_start(out=o_d[r0 : r0 + P, 2048:4096], in_=out_t[:, 2048:4096])
                        yield None

                    if braid:
                        if phase_bs:
                            for _ in phase_bs.pop(0):
                                pass
                        phase_bs.append(phase_b())
                    else:
                        phase_bs.append(phase_b())
                        for _ in phase_bs.pop(0):
                            pass
            while phase_bs:
                for _ in phase_bs.pop(0):
                    pass

    if split_waits:
        _split_waits(nc)
    return nc


_NC_CACHE = {}


def _get_nc(nrows):
    if nrows not in _NC_CACHE:
        _NC_CACHE[nrows] = build(nrows)
    return _NC_CACHE[nrows]


def make_aux(H):
    H32 = np.asarray(H, dtype=np.float32)
    hq = (np.sign(H32) * 0.125).astype(np.float16)
    hs = np.sign(H32).astype(np.float16)
    ident = np.eye(P, dtype=np.float16)
    onesc = np.ones((P, 1), dtype=np.float16)
    one1 = np.ones((1, 1), dtype=np.float32)
    return hq, hs, ident, onesc, one1


def block_transpose(xf16):
    """[rows, 4096] fp16 -> [NB, 128, rows] fp16 (pure layout)."""
    r = xf16.shape[0]
    return np.ascontiguousarray(
        xf16.reshape(r, NB, P).transpose(1, 2, 0)
    )


def kernel(x, H):
    x = np.asarray(x)
    orig_shape = x.shape
    xf = x.reshape(-1, NCOLS).astype(np.float16)
    nrows_total = xf.shape[0]
    assert nrows_total % N_CORES == 0
    shard = nrows_total // N_CORES

    hq, hs, ident, onesc, one1 = make_aux(H)
    nc = _get_nc(shard)

    in_maps = [
        {
            "x": block_transpose(xf[i * shard : (i + 1) * shard]),
            "hq": hq,
            "hs": hs,
            "ident": ident,
            "onesc": onesc,
            "one1": one1,
        }
        for i in range(N_CORES)
    ]
    res = run_bass_kernel_spmd(nc, in_maps, core_ids=list(range(N_CORES)))
    out = np.concatenate([r["o"] for r in res.results], axis=0)
    return out.reshape(orig_shape).astype(np.float32)


if __name__ == "__main__":
    rng = np.random.default_rng(0)
    nrows = 512
    x32 = rng.standard_normal((nrows, NCOLS), dtype=np.float32)

    Hnp = np.ones((1, 1))
    while Hnp.shape[0] < P:
        Hnp = np.block([[Hnp, Hnp], [Hnp, -Hnp]])
    Hnp = (Hnp / math.sqrt(P)).astype(np.float32)

    def ref(x, H):
        xr = (x.reshape(-1, NB, P) @ H).reshape(-1, NCOLS)
        std = np.maximum(np.sqrt((xr * xr).mean(-1, keepdims=True)), 1e-8)
        step = ALPHA * std / QMAX
        q = np.clip(np.round(xr / step), -QMAX, QMAX) * step
        return (q.reshape(-1, NB, P) @ H).reshape(-1, NCOLS)

    from concourse.bass_interp import CoreSim
    from concourse.timeline_sim import TimelineSim

    nc = build(nrows, split_waits=False)
    hq, hs, ident, onesc, one1 = make_aux(Hnp)
    x16 = x32.astype(np.float16)
    sim = CoreSim(nc)
    sim.tensor("x")[:] = block_transpose(x16)
    sim.tensor("hq")[:] = hq
    sim.tensor("hs")[:] = hs
    sim.tensor("ident")[:] = ident
    sim.tensor("onesc")[:] = onesc
    sim.tensor("one1")[:] = one1
    sim.simulate()
    got = np.asarray(sim.tensor("o")).astype(np.float32)
    want = ref(x32, Hnp)
    err = np.abs(got - want)
    denom = np.abs(want).max()
    l2 = np.linalg.norm((got - want).ravel()) / np.linalg.norm(want.ravel())
    print("max abs err:", err.max(), "rel l2:", l2)
    bad = (err > 1e-3 * denom).sum()
    print("elements off by >1e-3*scale:", bad, "/", err.size)

    nc2 = build(nrows)
    ts = TimelineSim(nc2)
    ts.simulate()
    print(f"timeline {nrows} rows: {ts.time:.0f}ns")


# revision 4
# speedup vs baseline: 1.0765x; 1.0452x over previous
"""HadamardTrustQuantizer Trainium2 kernel, v4 (host block-transposed input).

Forward math (mask term cancels):
    y   = blockwise_rot(x, H)          # H: 128x128 Hadamard
    std = max(sqrt(mean(y^2, -1)), 1e-8) = max(sqrt(mean(x^2, -1)), 1e-8)
    step = ALPHA*std/QMAX
    q   = clip(round(y/step), -QMAX, QMAX)
    out = blockwise_rot(q*step, H)

v4 strategy (per core, data-parallel shard of 2048 rows):
  - host hands x BLOCK-TRANSPOSED: x_fm[b, f, r] (pure layout change), fp16.
    This kills the on-device x-transposes and their PSUM drains.
  - row sum-of-squares: square chunks on DVE (2x), reduce over features with
    PE ones-matmuls accumulating into a [1, rows] PSUM strip; a K=1 fp32
    matmul transposes the strip to [rows, 1] for per-partition use.
  - mm1 row-major: lhsT = x-strip (stationary), rhs = Hq = sign(H)/8 moving;
    output y'' lands [row, feat] so the ACT drain fuses prescale + round:
    q' = fp16(y''*rs + 1536) = 1536 + round_even(u), rs per-partition.
  - un-offset + top clip fused into the q-transpose drain (DVE 2x):
    (q' - 1536) then min 7; bottom clip (max -7) one-op pass on GPSIMD.
  - mm2: stationary q (feature-major), moving hs = +-1 sign matrix; output
    row-major; ACT/DVE drain applies os = step/sqrt(128) per-partition.
"""

import math
import sys

sys.path.insert(0, "/opt/trn_rl_repo")

import ml_dtypes  # noqa: F401
import numpy as np

import concourse.bass as bass
import concourse.tile as tile
from concourse import mybir
from concourse.bass_utils import run_bass_kernel_spmd

P = 128
NCOLS = 4096
NB = NCOLS // P          # 32 blocks per row
ALPHA = 2.5139
QMAX = 7.0
OFF = 1536.0             # fp16 round-to-integer offset (ulp=1 in [1024,2048))
# s1 = sqrt(ssq * CS) = step*sqrt(2), where step = ALPHA*sqrt(ssq/4096)/QMAX
CS = (ALPHA * math.sqrt(2.0) / (QMAX * 64.0)) ** 2
# os = step/sqrt(128) = s1/16

N_CORES = 8
ROWS_PER_CORE = 2048
RCHUNK = 256             # rows per DMA chunk

F32 = mybir.dt.float32
F16 = mybir.dt.float16
Alu = mybir.AluOpType
Act = mybir.ActivationFunctionType


def _split_waits(nc, maxw_default=1, drain_maxw=1):
    """walrus in this container rejects >1 sem wait per instruction.
    Hoist excess waits onto preceding same-engine NoOps."""
    for bb in nc.m.functions[0].blocks:
        new_list, changed = [], False
        for inst in bb.instructions:
            si = inst.sync_info
            maxw = drain_maxw if type(inst).__name__ == "InstDrain" else maxw_default
            if si is not None and len(si.on_wait) > maxw:
                waits = list(si.on_wait)
                head, tail = waits[:-maxw], waits[-maxw:]
                k = 0
                while head:
                    chunk, head = head[:1], head[1:]
                    nop = mybir.InstNoOp(name=f"{inst.name}-ws{k}", ins=[], outs=[])
                    nop.engine = inst.engine
                    nop.sync_info = mybir.SyncInfo(on_wait=chunk, on_update=[])
                    new_list.append(nop)
                    k += 1
                inst.sync_info = mybir.SyncInfo(
                    on_wait=tail, on_update=list(si.on_update)
                )
                changed = True
            new_list.append(inst)
        if changed:
            bb.instructions = new_list


def build(nrows=ROWS_PER_CORE, split_waits=True,
          act_outdrains=(0, 1, 0, 1, 0, 1, 0, 1),
          alt_outdrains=(0, 1, 0, 1, 1, 1, 0, 1), braid=True):
    """Per-core program; input x_fm[NB, 128, nrows] fp16 (block-transposed)."""
    assert nrows % RCHUNK == 0
    n_chunks = nrows // RCHUNK
    tiles_per_chunk = RCHUNK // P
    n_tiles = nrows // P

    nc = bass.Bass("TRN2", target_bir_lowering=False)
    x_d = nc.dram_tensor("x", [NB, P, nrows], F16, kind="ExternalInput")
    hq_d = nc.dram_tensor("hq", [P, P], F16, kind="ExternalInput")
    hs_d = nc.dram_tensor("hs", [P, P], F16, kind="ExternalInput")
    id_d = nc.dram_tensor("ident", [P, P], F16, kind="ExternalInput")
    ones_d = nc.dram_tensor("onesc", [P, 1], F16, kind="ExternalInput")
    one1_d = nc.dram_tensor("one1", [1, 1], F32, kind="ExternalInput")
    o_d = nc.dram_tensor("o", [nrows, NCOLS], F16, kind="ExternalOutput")

    with tile.TileContext(nc) as tc:
        import contextlib

        with contextlib.ExitStack() as ctx:
            singles = ctx.enter_context(tc.tile_pool(name="singles", bufs=1))
            px = ctx.enter_context(tc.tile_pool(name="px", bufs=3))
            psq = ctx.enter_context(tc.tile_pool(name="psq", bufs=2))
            pq = ctx.enter_context(tc.tile_pool(name="pq", bufs=3))
            pqT = ctx.enter_context(tc.tile_pool(name="pqT", bufs=6))
            pqc = ctx.enter_context(tc.tile_pool(name="pqc", bufs=6))
            pout = ctx.enter_context(tc.tile_pool(name="pout", bufs=3))
            pst = ctx.enter_context(tc.tile_pool(name="pst", bufs=6))
            psT = ctx.enter_context(tc.tile_pool(name="psT", bufs=1, space="PSUM"))
            pyp = ctx.enter_context(tc.tile_pool(name="pyp", bufs=3, space="PSUM"))
            ptq = ctx.enter_context(tc.tile_pool(name="ptq", bufs=2, space="PSUM"))
            pop = ctx.enter_context(tc.tile_pool(name="pop", bufs=2, space="PSUM"))

            hq_sb = singles.tile([P, P], F16)
            hs_sb = singles.tile([P, P], F16)
            id_sb = singles.tile([P, P], F16)
            ones_sb = singles.tile([P, 1], F16)
            one1_sb = singles.tile([1, 1], F32)
            nc.sync.dma_start(out=hq_sb, in_=hq_d[:])
            nc.sync.dma_start(out=hs_sb, in_=hs_d[:])
            nc.sync.dma_start(out=id_sb, in_=id_d[:])
            nc.sync.dma_start(out=ones_sb, in_=ones_d[:])
            nc.sync.dma_start(out=one1_sb, in_=one1_d[:])

            x_chunks = {}

            def emit_dma(c):
                if c >= n_chunks:
                    return
                xs = px.tile([P, NB, RCHUNK], F16, tag="x")
                # 4 slab-aligned sub-DMAs so squares can start on slab 0
                # while the rest of the chunk is still in flight
                for s in range(4):
                    b0, b1 = s * (NB // 4), (s + 1) * (NB // 4)
                    src = x_d[b0:b1, :, c * RCHUNK : (c + 1) * RCHUNK]
                    nc.sync.dma_start(
                        out=xs[:, b0:b1, :],
                        in_=src.rearrange("b p r -> p b r"),
                    )
                x_chunks[c] = xs

            emit_dma(0)
            phase_bs = []

            for c in range(n_chunks):
                emit_dma(c + 1)
                xs = x_chunks.pop(c)

                # squares for the whole chunk (DVE 2x), in 4 wide slabs
                xsq = psq.tile([P, NB, RCHUNK], F16, tag="xsq")
                for s in range(4):
                    b0 = s * (NB // 4)
                    b1 = (s + 1) * (NB // 4)
                    nc.vector.tensor_tensor(
                        out=xsq[:, b0:b1, :], in0=xs[:, b0:b1, :],
                        in1=xs[:, b0:b1, :], op=Alu.mult,
                    )

                # stats for all tiles of the chunk, hoisted so rs/os are
                # ready well before the ACT rounds need them
                stats = []
                for t in range(tiles_per_chunk):
                    cs0 = t * P
                    # ssq[rows] directly as [128, 1]: stationary = xsq block,
                    # moving = ones column (N=1, nearly free on PE)
                    sqT = psT.tile([P, 1], F32, tag="sqT")
                    for b in range(NB):
                        nc.tensor.matmul(
                            sqT, lhsT=xsq[:, b, cs0 : cs0 + P],
                            rhs=ones_sb,
                            start=(b == 0), stop=(b == NB - 1),
                        )
                    # rs = 1/(step*sqrt(2)) = sqrt(1/(ssq*CS));
                    # os = step/sqrt(128) = sqrt(ssq*CS)/16
                    rcp = pst.tile([P, 1], F32, tag="rcp")
                    nc.vector.reciprocal(out=rcp, in_=sqT)
                    rs = pst.tile([P, 1], F32, tag="rs")
                    nc.scalar.activation(out=rs, in_=rcp, func=Act.Sqrt,
                                         scale=1.0 / CS)
                    os_t = pst.tile([P, 1], F32, tag="os")
                    nc.scalar.activation(out=os_t, in_=sqT, func=Act.Sqrt,
                                         scale=CS / 256.0)
                    stats.append((rs, os_t))

                for t in range(tiles_per_chunk):
                    rt = c * tiles_per_chunk + t
                    r0 = rt * P
                    cs0 = t * P
                    rs, os_t = stats[t]
                    od_mask = act_outdrains if rt % 2 == 0 else alt_outdrains

                    # mm1 row-major + fused ACT prescale/round/offset
                    q = pq.tile([P, NCOLS], F16, tag="q")
                    for g in range(8):
                        yp = pyp.tile([P, 512], F32, tag="yp")
                        for j in range(4):
                            b = 4 * g + j
                            nc.tensor.matmul(
                                yp[:, j * P : (j + 1) * P],
                                lhsT=xs[:, b, cs0 : cs0 + P],
                                rhs=hq_sb,
                                start=True, stop=True,
                            )
                        nc.scalar.activation(
                            out=q[:, g * 512 : (g + 1) * 512], in_=yp,
                            func=Act.Copy, scale=rs[:, 0:1], bias=OFF,
                        )

                    def phase_b(q=q, os_t=os_t, r0=r0):
                        out_t = pout.tile([P, NCOLS], F16, tag="out")
                        for g in range(4):
                            tq = ptq.tile([P, 1024], F16, tag="tq")
                            for k in range(8):
                                b = 8 * g + k
                                nc.tensor.transpose(
                                    tq[:, k * P : (k + 1) * P],
                                    q[:, b * P : (b + 1) * P],
                                    id_sb,
                                )
                            qT = pqT.tile([P, 1024], F16, tag="qT")
                            nc.vector.tensor_scalar(
                                out=qT, in0=tq, scalar1=OFF, scalar2=QMAX,
                                op0=Alu.subtract, op1=Alu.min,
                            )
                            qc = pqc.tile([P, 1024], F16, tag="qc")
                            nc.gpsimd.tensor_scalar(
                                out=qc, in0=qT, scalar1=-QMAX, scalar2=None,
                                op0=Alu.max,
                            )
                            for h in range(2):
                                bank = 2 * g + h
                                op_t = pop.tile([P, 512], F32, tag="op")
                                for j in range(4):
                                    k = h * 4 + j
                                    nc.tensor.matmul(
                                        op_t[:, j * P : (j + 1) * P],
                                        lhsT=qc[:, k * P : (k + 1) * P],
                                        rhs=hs_sb,
                                        start=True, stop=True,
                                    )
                                c0 = g * 1024 + h * 512
                                if od_mask[bank]:
                                    nc.scalar.activation(
                                        out=out_t[:, c0 : c0 + 512], in_=op_t,
                                        func=Act.Copy, scale=os_t[:, 0:1],
                                    )
                                else:
                                    nc.vector.tensor_scalar(
                                        out=out_t[:, c0 : c0 + 512], in0=op_t,
                                        scalar1=os_t[:, 0:1], scalar2=None,
                                        op0=Alu.mult,
                                    )
                            yield None
                        nc.sync.dma_start(out=o_d[r0 : r0 + P, 0:2048], in_=out_t[:, 0:2048])
                        nc.sync.dma_start(out=o_d[r0 : r0 + P, 2048:4096], in_=out_t[:, 2048:4096])
                        yield None

                    if braid:
                        if phase_bs:
                            for _ in phase_bs.pop(0):
                                pass
                        phase_bs.append(phase_b())
                    else:
                        phase_bs.append(phase_b())
                        for _ in phase_bs.pop(0):
                            pass
            while phase_bs:
                for _ in phase_bs.pop(0):
                    pass

    if split_waits:
        _split_waits(nc)
    return nc


_NC_CACHE = {}


def _get_nc(nrows):
    if nrows not in _NC_CACHE:
        _NC_CACHE[nrows] = build(nrows)
    return _NC_CACHE[nrows]


def make_aux(H):
    H32 = np.asarray(H, dtype=np.float32)
    hq = (np.sign(H32) * 0.125).astype(np.float16)
    hs = np.sign(H32).astype(np.float16)
    ident = np.eye(P, dtype=np.float16)
    onesc = np.ones((P, 1), dtype=np.float16)
    one1 = np.ones((1, 1), dtype=np.float32)
    return hq, hs, ident, onesc, one1


def block_transpose(xf16):
    """[rows, 4096] fp16 -> [NB, 128, rows] fp16 (pure layout)."""
    r = xf16.shape[0]
    return np.ascontiguousarray(
        xf16.reshape(r, NB, P).transpose(1, 2, 0)
    )


def kernel(x, H):
    x = np.asarray(x)
    orig_shape = x.shape
    xf = x.reshape(-1, NCOLS).astype(np.float16)
    nrows_total = xf.shape[0]
    assert nrows_total % N_CORES == 0
    shard = nrows_total // N_CORES

    hq, hs, ident, onesc, one1 = make_aux(H)
    nc = _get_nc(shard)

    in_maps = [
        {
            "x": block_transpose(xf[i * shard : (i + 1) * shard]),
            "hq": hq,
            "hs": hs,
            "ident": ident,
            "onesc": onesc,
            "one1": one1,
        }
        for i in range(N_CORES)
    ]
    res = run_bass_kernel_spmd(nc, in_maps, core_ids=list(range(N_CORES)))
    out = np.concatenate([r["o"] for r in res.results], axis=0)
    return out.reshape(orig_shape).astype(np.float32)


if __name__ == "__main__":
    rng = np.random.default_rng(0)
    nrows = 512
    x32 = rng.standard_normal((nrows, NCOLS), dtype=np.float32)

    Hnp = np.ones((1, 1))
    while Hnp.shape[0] < P:
        Hnp = np.block([[Hnp, Hnp], [Hnp, -Hnp]])
    Hnp = (Hnp / math.sqrt(P)).astype(np.float32)

    def ref(x, H):
        xr = (x.reshape(-1, NB, P) @ H).reshape(-1, NCOLS)
        std = np.maximum(np.sqrt((xr * xr).mean(-1, keepdims=True)), 1e-8)
        step = ALPHA * std / QMAX
        q = np.clip(np.round(xr / step), -QMAX, QMAX) * step
        return (q.reshape(-1, NB, P) @ H).reshape(-1, NCOLS)

    from concourse.bass_interp import CoreSim
    from concourse.timeline_sim import TimelineSim

    nc = build(nrows, split_waits=False)
    hq, hs, ident, onesc, one1 = make_aux(Hnp)
    x16 = x32.astype(np.float16)
    sim = CoreSim(nc)
    sim.tensor("x")[:] = block_transpose(x16)
    sim.tensor("hq")[:] = hq
    sim.tensor("hs")[:] = hs
    sim.tensor("ident")[:] = ident
    sim.tensor("onesc")[:] = onesc
    sim.tensor("one1")[:] = one1
    sim.simulate()
    got = np.asarray(sim.tensor("o")).astype(np.float32)
    want = ref(x32, Hnp)
    err = np.abs(got - want)
    denom = np.abs(want).max()
    l2 = np.linalg.norm((got - want).ravel()) / np.linalg.norm(want.ravel())
    print("max abs err:", err.max(), "rel l2:", l2)
    bad = (err > 1e-3 * denom).sum()
    print("elements off by >1e-3*scale:", bad, "/", err.size)

    nc2 = build(nrows)
    ts = TimelineSim(nc2)
    ts.simulate()
    print(f"timeline {nrows} rows: {ts.time:.0f}ns")


# revision 5
# speedup vs baseline: 1.0862x; 1.0090x over previous
"""HadamardTrustQuantizer Trainium2 kernel, v4 (host block-transposed input).

Forward math (mask term cancels):
    y   = blockwise_rot(x, H)          # H: 128x128 Hadamard
    std = max(sqrt(mean(y^2, -1)), 1e-8) = max(sqrt(mean(x^2, -1)), 1e-8)
    step = ALPHA*std/QMAX
    q   = clip(round(y/step), -QMAX, QMAX)
    out = blockwise_rot(q*step, H)

v4 strategy (per core, data-parallel shard of 2048 rows):
  - host hands x BLOCK-TRANSPOSED: x_fm[b, f, r] (pure layout change), fp16.
    This kills the on-device x-transposes and their PSUM drains.
  - row sum-of-squares: square chunks on DVE (2x), reduce over features with
    PE ones-matmuls accumulating into a [1, rows] PSUM strip; a K=1 fp32
    matmul transposes the strip to [rows, 1] for per-partition use.
  - mm1 row-major: lhsT = x-strip (stationary), rhs = Hq = sign(H)/8 moving;
    output y'' lands [row, feat] so the ACT drain fuses prescale + round:
    q' = fp16(y''*rs + 1536) = 1536 + round_even(u), rs per-partition.
  - un-offset + top clip fused into the q-transpose drain (DVE 2x):
    (q' - 1536) then min 7; bottom clip (max -7) one-op pass on GPSIMD.
  - mm2: stationary q (feature-major), moving hs = +-1 sign matrix; output
    row-major; ACT/DVE drain applies os = step/sqrt(128) per-partition.
"""

import math
import sys

sys.path.insert(0, "/opt/trn_rl_repo")

import ml_dtypes  # noqa: F401
import numpy as np

import concourse.bass as bass
import concourse.tile as tile
from concourse import mybir
from concourse.bass_utils import run_bass_kernel_spmd

P = 128
NCOLS = 4096
NB = NCOLS // P          # 32 blocks per row
ALPHA = 2.5139
QMAX = 7.0
OFF = 1536.0             # fp16 round-to-integer offset (ulp=1 in [1024,2048))
# s1 = sqrt(ssq * CS) = step*sqrt(2), where step = ALPHA*sqrt(ssq/4096)/QMAX
CS = (ALPHA * math.sqrt(2.0) / (QMAX * 64.0)) ** 2
# os = step/sqrt(128) = s1/16

N_CORES = 8
ROWS_PER_CORE = 2048
RCHUNK = 256             # rows per DMA chunk

F32 = mybir.dt.float32
F16 = mybir.dt.float16
Alu = mybir.AluOpType
Act = mybir.ActivationFunctionType


def _split_waits(nc, maxw_default=1, drain_maxw=1):
    """walrus in this container rejects >1 sem wait per instruction.
    Hoist excess waits onto preceding same-engine NoOps."""
    for bb in nc.m.functions[0].blocks:
        new_list, changed = [], False
        for inst in bb.instructions:
            si = inst.sync_info
            maxw = drain_maxw if type(inst).__name__ == "InstDrain" else maxw_default
            if si is not None and len(si.on_wait) > maxw:
                waits = list(si.on_wait)
                head, tail = waits[:-maxw], waits[-maxw:]
                k = 0
                while head:
                    chunk, head = head[:1], head[1:]
                    nop = mybir.InstNoOp(name=f"{inst.name}-ws{k}", ins=[], outs=[])
                    nop.engine = inst.engine
                    nop.sync_info = mybir.SyncInfo(on_wait=chunk, on_update=[])
                    new_list.append(nop)
                    k += 1
                inst.sync_info = mybir.SyncInfo(
                    on_wait=tail, on_update=list(si.on_update)
                )
                changed = True
            new_list.append(inst)
        if changed:
            bb.instructions = new_list


def build(nrows=ROWS_PER_CORE, split_waits=True,
          act_outdrains=(0, 1, 0, 1, 0, 1, 0, 1),
          alt_outdrains=(0, 1, 0, 1, 1, 1, 0, 1), braid=True):
    """Per-core program; input x_fm[NB, 128, nrows] fp16 (block-transposed)."""
    assert nrows % RCHUNK == 0
    n_chunks = nrows // RCHUNK
    tiles_per_chunk = RCHUNK // P
    n_tiles = nrows // P

    nc = bass.Bass("TRN2", target_bir_lowering=False)
    x_d = nc.dram_tensor("x", [NB, P, nrows], F16, kind="ExternalInput")
    hq_d = nc.dram_tensor("hq", [P, P], F16, kind="ExternalInput")
    hs_d = nc.dram_tensor("hs", [P, P], F16, kind="ExternalInput")
    id_d = nc.dram_tensor("ident", [P, P], F16, kind="ExternalInput")
    ones_d = nc.dram_tensor("onesc", [P, 1], F16, kind="ExternalInput")
    one1_d = nc.dram_tensor("one1", [1, 1], F32, kind="ExternalInput")
    o_d = nc.dram_tensor("o", [nrows, NCOLS], F16, kind="ExternalOutput")

    with tile.TileContext(nc) as tc:
        import contextlib

        with contextlib.ExitStack() as ctx:
            singles = ctx.enter_context(tc.tile_pool(name="singles", bufs=1))
            px = ctx.enter_context(tc.tile_pool(name="px", bufs=3))
            psq = ctx.enter_context(tc.tile_pool(name="psq", bufs=2))
            pq = ctx.enter_context(tc.tile_pool(name="pq", bufs=3))
            pqT = ctx.enter_context(tc.tile_pool(name="pqT", bufs=6))
            pqc = ctx.enter_context(tc.tile_pool(name="pqc", bufs=6))
            pout = ctx.enter_context(tc.tile_pool(name="pout", bufs=3))
            pst = ctx.enter_context(tc.tile_pool(name="pst", bufs=6))
            psT = ctx.enter_context(tc.tile_pool(name="psT", bufs=1, space="PSUM"))
            pyp = ctx.enter_context(tc.tile_pool(name="pyp", bufs=3, space="PSUM"))
            ptq = ctx.enter_context(tc.tile_pool(name="ptq", bufs=2, space="PSUM"))
            pop = ctx.enter_context(tc.tile_pool(name="pop", bufs=2, space="PSUM"))

            hq_sb = singles.tile([P, P], F16)
            hs_sb = singles.tile([P, P], F16)
            id_sb = singles.tile([P, P], F16)
            ones_sb = singles.tile([P, 1], F16)
            one1_sb = singles.tile([1, 1], F32)
            nc.sync.dma_start(out=hq_sb, in_=hq_d[:])
            nc.sync.dma_start(out=hs_sb, in_=hs_d[:])
            nc.sync.dma_start(out=id_sb, in_=id_d[:])
            nc.sync.dma_start(out=ones_sb, in_=ones_d[:])
            nc.sync.dma_start(out=one1_sb, in_=one1_d[:])

            x_chunks = {}

            def emit_dma(c):
                if c >= n_chunks:
                    return
                xs = px.tile([P, NB, RCHUNK], F16, tag="x")
                # 4 slab-aligned sub-DMAs so squares can start on slab 0
                # while the rest of the chunk is still in flight
                for s in range(4):
                    b0, b1 = s * (NB // 4), (s + 1) * (NB // 4)
                    src = x_d[b0:b1, :, c * RCHUNK : (c + 1) * RCHUNK]
                    nc.sync.dma_start(
                        out=xs[:, b0:b1, :],
                        in_=src.rearrange("b p r -> p b r"),
                    )
                x_chunks[c] = xs

            emit_dma(0)
            phase_bs = []

            for c in range(n_chunks):
                emit_dma(c + 1)
                xs = x_chunks.pop(c)

                # squares for the whole chunk (DVE 2x), in 4 wide slabs
                xsq = psq.tile([P, NB, RCHUNK], F16, tag="xsq")
                for s in range(4):
                    b0 = s * (NB // 4)
                    b1 = (s + 1) * (NB // 4)
                    nc.vector.tensor_tensor(
                        out=xsq[:, b0:b1, :], in0=xs[:, b0:b1, :],
                        in1=xs[:, b0:b1, :], op=Alu.mult,
                    )

                # stats for all tiles of the chunk, hoisted so rs/os are
                # ready well before the ACT rounds need them
                stats = []
                for t in range(tiles_per_chunk):
                    cs0 = t * P
                    # ssq[rows] directly as [128, 1]: stationary = xsq block,
                    # moving = ones column (N=1, nearly free on PE)
                    sqT = psT.tile([P, 1], F32, tag="sqT")
                    for b in range(NB):
                        nc.tensor.matmul(
                            sqT, lhsT=xsq[:, b, cs0 : cs0 + P],
                            rhs=ones_sb,
                            start=(b == 0), stop=(b == NB - 1),
                        )
                    # rs = 1/(step*sqrt(2)) = sqrt(1/(ssq*CS));
                    # os = step/sqrt(128) = sqrt(ssq*CS)/16
                    rcp = pst.tile([P, 1], F32, tag="rcp")
                    nc.vector.reciprocal(out=rcp, in_=sqT)
                    rs = pst.tile([P, 1], F32, tag="rs")
                    nc.scalar.activation(out=rs, in_=rcp, func=Act.Sqrt,
                                         scale=1.0 / CS)
                    os_t = pst.tile([P, 1], F32, tag="os")
                    nc.scalar.activation(out=os_t, in_=sqT, func=Act.Sqrt,
                                         scale=CS / 256.0)
                    stats.append((rs, os_t))

                for t in range(tiles_per_chunk):
                    rt = c * tiles_per_chunk + t
                    r0 = rt * P
                    cs0 = t * P
                    rs, os_t = stats[t]
                    od_mask = act_outdrains if rt % 2 == 0 else alt_outdrains

                    # mm1 row-major + fused ACT prescale/round/offset
                    q = pq.tile([P, NCOLS], F16, tag="q")
                    for g in range(8):
                        yp = pyp.tile([P, 512], F32, tag="yp")
                        for j in range(4):
                            b = 4 * g + j
                            nc.tensor.matmul(
                                yp[:, j * P : (j + 1) * P],
                                lhsT=xs[:, b, cs0 : cs0 + P],
                                rhs=hq_sb,
                                start=True, stop=True,
                            )
                        nc.scalar.activation(
                            out=q[:, g * 512 : (g + 1) * 512], in_=yp,
                            func=Act.Copy, scale=rs[:, 0:1], bias=OFF,
                        )
                        if braid and phase_bs and g in (3, 7):
                            next(phase_bs[0], None)

                    def phase_b(q=q, os_t=os_t, r0=r0):
                        out_t = pout.tile([P, NCOLS], F16, tag="out")
                        for g in range(4):
                            tq = ptq.tile([P, 1024], F16, tag="tq")
                            for k in range(8):
                                b = 8 * g + k
                                nc.tensor.transpose(
                                    tq[:, k * P : (k + 1) * P],
                                    q[:, b * P : (b + 1) * P],
                                    id_sb,
                                )
                            qT = pqT.tile([P, 1024], F16, tag="qT")
                            nc.vector.tensor_scalar(
                                out=qT, in0=tq, scalar1=OFF, scalar2=QMAX,
                                op0=Alu.subtract, op1=Alu.min,
                            )
                            qc = pqc.tile([P, 1024], F16, tag="qc")
                            nc.gpsimd.tensor_scalar(
                                out=qc, in0=qT, scalar1=-QMAX, scalar2=None,
                                op0=Alu.max,
                            )
                            for h in range(2):
                                bank = 2 * g + h
                                op_t = pop.tile([P, 512], F32, tag="op")
                                for j in range(4):
                                    k = h * 4 + j
                                    nc.tensor.matmul(
                                        op_t[:, j * P : (j + 1) * P],
                                        lhsT=qc[:, k * P : (k + 1) * P],
                                        rhs=hs_sb,
                                        start=True, stop=True,
                                    )
                                c0 = g * 1024 + h * 512
                                if od_mask[bank]:
                                    nc.scalar.activation(
                                        out=out_t[:, c0 : c0 + 512], in_=op_t,
                                        func=Act.Copy, scale=os_t[:, 0:1],
                                    )
                                else:
                                    nc.vector.tensor_scalar(
                                        out=out_t[:, c0 : c0 + 512], in0=op_t,
                                        scalar1=os_t[:, 0:1], scalar2=None,
                                        op0=Alu.mult,
                                    )
                            yield None
                        nc.sync.dma_start(out=o_d[r0 : r0 + P, 0:2048], in_=out_t[:, 0:2048])
                        nc.sync.dma_start(out=o_d[r0 : r0 + P, 2048:4096], in_=out_t[:, 2048:4096])
                        yield None

                    if braid:
                        if phase_bs:
                            for _ in phase_bs.pop(0):
                                pass
                        phase_bs.append(phase_b())
                    else:
                        phase_bs.append(phase_b())
                        for _ in phase_bs.pop(0):
                            pass
            while phase_bs:
                for _ in phase_bs.pop(0):
                    pass

    if split_waits:
        _split_waits(nc)
    return nc


_NC_CACHE = {}


def _get_nc(nrows):
    if nrows not in _NC_CACHE:
        _NC_CACHE[nrows] = build(nrows)
    return _NC_CACHE[nrows]


def make_aux(H):
    H32 = np.asarray(H, dtype=np.float32)
    hq = (np.sign(H32) * 0.125).astype(np.float16)
    hs = np.sign(H32).astype(np.float16)
    ident = np.eye(P, dtype=np.float16)
    onesc = np.ones((P, 1), dtype=np.float16)
    one1 = np.ones((1, 1), dtype=np.float32)
    return hq, hs, ident, onesc, one1


def block_transpose(xf16):
    """[rows, 4096] fp16 -> [NB, 128, rows] fp16 (pure layout)."""
    r = xf16.shape[0]
    return np.ascontiguousarray(
        xf16.reshape(r, NB, P).transpose(1, 2, 0)
    )


def kernel(x, H):
    x = np.asarray(x)
    orig_shape = x.shape
    xf = x.reshape(-1, NCOLS).astype(np.float16)
    nrows_total = xf.shape[0]
    assert nrows_total % N_CORES == 0
    shard = nrows_total // N_CORES

    hq, hs, ident, onesc, one1 = make_aux(H)
    nc = _get_nc(shard)

    in_maps = [
        {
            "x": block_transpose(xf[i * shard : (i + 1) * shard]),
            "hq": hq,
            "hs": hs,
            "ident": ident,
            "onesc": onesc,
            "one1": one1,
        }
        for i in range(N_CORES)
    ]
    res = run_bass_kernel_spmd(nc, in_maps, core_ids=list(range(N_CORES)))
    out = np.concatenate([r["o"] for r in res.results], axis=0)
    return out.reshape(orig_shape).astype(np.float32)


if __name__ == "__main__":
    rng = np.random.default_rng(0)
    nrows = 512
    x32 = rng.standard_normal((nrows, NCOLS), dtype=np.float32)

    Hnp = np.ones((1, 1))
    while Hnp.shape[0] < P:
        Hnp = np.block([[Hnp, Hnp], [Hnp, -Hnp]])
    Hnp = (Hnp / math.sqrt(P)).astype(np.float32)

    def ref(x, H):
        xr = (x.reshape(-1, NB, P) @ H).reshape(-1, NCOLS)
        std = np.maximum(np.sqrt((xr * xr).mean(-1, keepdims=True)), 1e-8)
        step = ALPHA * std / QMAX
        q = np.clip(np.round(xr / step), -QMAX, QMAX) * step
        return (q.reshape(-1, NB, P) @ H).reshape(-1, NCOLS)

    from concourse.bass_interp import CoreSim
    from concourse.timeline_sim import TimelineSim

    nc = build(nrows, split_waits=False)
    hq, hs, ident, onesc, one1 = make_aux(Hnp)
    x16 = x32.astype(np.float16)
    sim = CoreSim(nc)
    sim.tensor("x")[:] = block_transpose(x16)
    sim.tensor("hq")[:] = hq
    sim.tensor("hs")[:] = hs
    sim.tensor("ident")[:] = ident
    sim.tensor("onesc")[:] = onesc
    sim.tensor("one1")[:] = one1
    sim.simulate()
    got = np.asarray(sim.tensor("o")).astype(np.float32)
    want = ref(x32, Hnp)
    err = np.abs(got - want)
    denom = np.abs(want).max()
    l2 = np.linalg.norm((got - want).ravel()) / np.linalg.norm(want.ravel())
    print("max abs err:", err.max(), "rel l2:", l2)
    bad = (err > 1e-3 * denom).sum()
    print("elements off by >1e-3*scale:", bad, "/", err.size)

    nc2 = build(nrows)
    ts = TimelineSim(nc2)
    ts.simulate()
    print(f"timeline {nrows} rows: {ts.time:.0f}ns")


# revision 6
# speedup vs baseline: 1.0885x; 1.0021x over previous
"""HadamardTrustQuantizer Trainium2 kernel, v4 (host block-transposed input).

Forward math (mask term cancels):
    y   = blockwise_rot(x, H)          # H: 128x128 Hadamard
    std = max(sqrt(mean(y^2, -1)), 1e-8) = max(sqrt(mean(x^2, -1)), 1e-8)
    step = ALPHA*std/QMAX
    q   = clip(round(y/step), -QMAX, QMAX)
    out = blockwise_rot(q*step, H)

v4 strategy (per core, data-parallel shard of 2048 rows):
  - host hands x BLOCK-TRANSPOSED: x_fm[b, f, r] (pure layout change), fp16.
    This kills the on-device x-transposes and their PSUM drains.
  - row sum-of-squares: square chunks on DVE (2x), reduce over features with
    PE ones-matmuls accumulating into a [1, rows] PSUM strip; a K=1 fp32
    matmul transposes the strip to [rows, 1] for per-partition use.
  - mm1 row-major: lhsT = x-strip (stationary), rhs = Hq = sign(H)/8 moving;
    output y'' lands [row, feat] so the ACT drain fuses prescale + round:
    q' = fp16(y''*rs + 1536) = 1536 + round_even(u), rs per-partition.
  - un-offset + top clip fused into the q-transpose drain (DVE 2x):
    (q' - 1536) then min 7; bottom clip (max -7) one-op pass on GPSIMD.
  - mm2: stationary q (feature-major), moving hs = +-1 sign matrix; output
    row-major; ACT/DVE drain applies os = step/sqrt(128) per-partition.
"""

import math
import sys

sys.path.insert(0, "/opt/trn_rl_repo")

import ml_dtypes  # noqa: F401
import numpy as np

import concourse.bass as bass
import concourse.tile as tile
from concourse import mybir
from concourse.bass_utils import run_bass_kernel_spmd

P = 128
NCOLS = 4096
NB = NCOLS // P          # 32 blocks per row
ALPHA = 2.5139
QMAX = 7.0
OFF = 1536.0             # fp16 round-to-integer offset (ulp=1 in [1024,2048))
# s1 = sqrt(ssq * CS) = step*sqrt(2), where step = ALPHA*sqrt(ssq/4096)/QMAX
CS = (ALPHA * math.sqrt(2.0) / (QMAX * 64.0)) ** 2
# os = step/sqrt(128) = s1/16

N_CORES = 8
ROWS_PER_CORE = 2048
RCHUNK = 256             # rows per DMA chunk

F32 = mybir.dt.float32
F16 = mybir.dt.float16
Alu = mybir.AluOpType
Act = mybir.ActivationFunctionType


def _split_waits(nc, maxw_default=1, drain_maxw=1):
    """walrus in this container rejects >1 sem wait per instruction.
    Hoist excess waits onto preceding same-engine NoOps."""
    for bb in nc.m.functions[0].blocks:
        new_list, changed = [], False
        for inst in bb.instructions:
            si = inst.sync_info
            maxw = drain_maxw if type(inst).__name__ == "InstDrain" else maxw_default
            if si is not None and len(si.on_wait) > maxw:
                waits = list(si.on_wait)
                head, tail = waits[:-maxw], waits[-maxw:]
                k = 0
                while head:
                    chunk, head = head[:1], head[1:]
                    nop = mybir.InstNoOp(name=f"{inst.name}-ws{k}", ins=[], outs=[])
                    nop.engine = inst.engine
                    nop.sync_info = mybir.SyncInfo(on_wait=chunk, on_update=[])
                    new_list.append(nop)
                    k += 1
                inst.sync_info = mybir.SyncInfo(
                    on_wait=tail, on_update=list(si.on_update)
                )
                changed = True
            new_list.append(inst)
        if changed:
            bb.instructions = new_list


def build(nrows=ROWS_PER_CORE, split_waits=True,
          act_outdrains=(0, 1, 0, 1, 0, 1, 0, 1),
          alt_outdrains=(0, 1, 0, 1, 1, 1, 0, 1), braid=True):
    """Per-core program; input x_fm[NB, 128, nrows] fp16 (block-transposed)."""
    assert nrows % RCHUNK == 0
    n_chunks = nrows // RCHUNK
    tiles_per_chunk = RCHUNK // P
    n_tiles = nrows // P

    nc = bass.Bass("TRN2", target_bir_lowering=False)
    x_d = nc.dram_tensor("x", [NB, P, nrows], F16, kind="ExternalInput")
    hq_d = nc.dram_tensor("hq", [P, P], F16, kind="ExternalInput")
    hs_d = nc.dram_tensor("hs", [P, P], F16, kind="ExternalInput")
    id_d = nc.dram_tensor("ident", [P, P], F16, kind="ExternalInput")
    ones_d = nc.dram_tensor("onesc", [P, 1], F16, kind="ExternalInput")
    one1_d = nc.dram_tensor("one1", [1, 1], F32, kind="ExternalInput")
    o_d = nc.dram_tensor("o", [nrows, NCOLS], F16, kind="ExternalOutput")

    with tile.TileContext(nc) as tc:
        import contextlib

        with contextlib.ExitStack() as ctx:
            singles = ctx.enter_context(tc.tile_pool(name="singles", bufs=1))
            px = ctx.enter_context(tc.tile_pool(name="px", bufs=3))
            psq = ctx.enter_context(tc.tile_pool(name="psq", bufs=2))
            pq = ctx.enter_context(tc.tile_pool(name="pq", bufs=3))
            pqT = ctx.enter_context(tc.tile_pool(name="pqT", bufs=6))
            pqc = ctx.enter_context(tc.tile_pool(name="pqc", bufs=6))
            pout = ctx.enter_context(tc.tile_pool(name="pout", bufs=3))
            pst = ctx.enter_context(tc.tile_pool(name="pst", bufs=6))
            psT = ctx.enter_context(tc.tile_pool(name="psT", bufs=1, space="PSUM"))
            pyp = ctx.enter_context(tc.tile_pool(name="pyp", bufs=3, space="PSUM"))
            ptq = ctx.enter_context(tc.tile_pool(name="ptq", bufs=2, space="PSUM"))
            pop = ctx.enter_context(tc.tile_pool(name="pop", bufs=2, space="PSUM"))

            hq_sb = singles.tile([P, P], F16)
            hs_sb = singles.tile([P, P], F16)
            id_sb = singles.tile([P, P], F16)
            ones_sb = singles.tile([P, 1], F16)
            one1_sb = singles.tile([1, 1], F32)
            nc.sync.dma_start(out=hq_sb, in_=hq_d[:])
            nc.sync.dma_start(out=hs_sb, in_=hs_d[:])
            nc.sync.dma_start(out=id_sb, in_=id_d[:])
            nc.sync.dma_start(out=ones_sb, in_=ones_d[:])
            nc.sync.dma_start(out=one1_sb, in_=one1_d[:])

            x_chunks = {}

            def emit_dma(c):
                if c >= n_chunks:
                    return
                xs = px.tile([P, NB, RCHUNK], F16, tag="x")
                # 4 slab-aligned sub-DMAs so squares can start on slab 0
                # while the rest of the chunk is still in flight
                for s in range(4):
                    b0, b1 = s * (NB // 4), (s + 1) * (NB // 4)
                    src = x_d[b0:b1, :, c * RCHUNK : (c + 1) * RCHUNK]
                    nc.sync.dma_start(
                        out=xs[:, b0:b1, :],
                        in_=src.rearrange("b p r -> p b r"),
                    )
                x_chunks[c] = xs

            emit_dma(0)
            phase_bs = []

            for c in range(n_chunks):
                emit_dma(c + 1)
                xs = x_chunks.pop(c)

                # squares for the whole chunk (DVE 2x), in 4 wide slabs
                xsq = psq.tile([P, NB, RCHUNK], F16, tag="xsq")
                for s in range(4):
                    b0 = s * (NB // 4)
                    b1 = (s + 1) * (NB // 4)
                    nc.vector.tensor_tensor(
                        out=xsq[:, b0:b1, :], in0=xs[:, b0:b1, :],
                        in1=xs[:, b0:b1, :], op=Alu.mult,
                    )

                # stats for all tiles of the chunk, hoisted so rs/os are
                # ready well before the ACT rounds need them
                stats = []
                for t in range(tiles_per_chunk):
                    cs0 = t * P
                    # ssq[rows] directly as [128, 1]: stationary = xsq block,
                    # moving = ones column (N=1, nearly free on PE)
                    sqT = psT.tile([P, 1], F32, tag="sqT")
                    for b in range(NB):
                        nc.tensor.matmul(
                            sqT, lhsT=xsq[:, b, cs0 : cs0 + P],
                            rhs=ones_sb,
                            start=(b == 0), stop=(b == NB - 1),
                        )
                    # rs = 1/(step*sqrt(2)) = sqrt(1/(ssq*CS));
                    # os = step/sqrt(128) = sqrt(ssq*CS)/16
                    rcp = pst.tile([P, 1], F32, tag="rcp")
                    nc.vector.reciprocal(out=rcp, in_=sqT)
                    rs = pst.tile([P, 1], F32, tag="rs")
                    nc.scalar.activation(out=rs, in_=rcp, func=Act.Sqrt,
                                         scale=1.0 / CS)
                    os_t = pst.tile([P, 1], F32, tag="os")
                    nc.scalar.activation(out=os_t, in_=sqT, func=Act.Sqrt,
                                         scale=CS / 256.0)
                    stats.append((rs, os_t))

                for t in range(tiles_per_chunk):
                    rt = c * tiles_per_chunk + t
                    r0 = rt * P
                    cs0 = t * P
                    rs, os_t = stats[t]
                    od_mask = act_outdrains if rt % 2 == 0 else alt_outdrains

                    # mm1 row-major + fused ACT prescale/round/offset
                    q = pq.tile([P, NCOLS], F16, tag="q")
                    for g in range(8):
                        yp = pyp.tile([P, 512], F32, tag="yp")
                        for j in range(4):
                            b = 4 * g + j
                            nc.tensor.matmul(
                                yp[:, j * P : (j + 1) * P],
                                lhsT=xs[:, b, cs0 : cs0 + P],
                                rhs=hq_sb,
                                start=True, stop=True,
                            )
                        nc.scalar.activation(
                            out=q[:, g * 512 : (g + 1) * 512], in_=yp,
                            func=Act.Copy, scale=rs[:, 0:1], bias=OFF,
                        )
                        if braid and phase_bs and g in (3, 7):
                            next(phase_bs[0], None)

                    def phase_b(q=q, os_t=os_t, r0=r0):
                        out_t = pout.tile([P, NCOLS], F16, tag="out")
                        for g in range(4):
                            tq = ptq.tile([P, 1024], F16, tag="tq")
                            for k in range(8):
                                b = 8 * g + k
                                nc.tensor.transpose(
                                    tq[:, k * P : (k + 1) * P],
                                    q[:, b * P : (b + 1) * P],
                                    id_sb,
                                )
                            qT = pqT.tile([P, 1024], F16, tag="qT")
                            nc.vector.tensor_scalar(
                                out=qT, in0=tq, scalar1=OFF, scalar2=QMAX,
                                op0=Alu.subtract, op1=Alu.min,
                            )
                            qc = pqc.tile([P, 1024], F16, tag="qc")
                            for hh in range(2):
                                nc.gpsimd.tensor_scalar(
                                    out=qc[:, hh * 512 : (hh + 1) * 512],
                                    in0=qT[:, hh * 512 : (hh + 1) * 512],
                                    scalar1=-QMAX, scalar2=None,
                                    op0=Alu.max,
                                )
                            for h in range(2):
                                bank = 2 * g + h
                                op_t = pop.tile([P, 512], F32, tag="op")
                                for j in range(4):
                                    k = h * 4 + j
                                    nc.tensor.matmul(
                                        op_t[:, j * P : (j + 1) * P],
                                        lhsT=qc[:, k * P : (k + 1) * P],
                                        rhs=hs_sb,
                                        start=True, stop=True,
                                    )
                                c0 = g * 1024 + h * 512
                                if od_mask[bank]:
                                    nc.scalar.activation(
                                        out=out_t[:, c0 : c0 + 512], in_=op_t,
                                        func=Act.Copy, scale=os_t[:, 0:1],
                                    )
                                else:
                                    nc.vector.tensor_scalar(
                                        out=out_t[:, c0 : c0 + 512], in0=op_t,
                                        scalar1=os_t[:, 0:1], scalar2=None,
                                        op0=Alu.mult,
                                    )
                            yield None
                        nc.sync.dma_start(out=o_d[r0 : r0 + P, 0:2048], in_=out_t[:, 0:2048])
                        nc.sync.dma_start(out=o_d[r0 : r0 + P, 2048:4096], in_=out_t[:, 2048:4096])
                        yield None

                    if braid:
                        if phase_bs:
                            for _ in phase_bs.pop(0):
                                pass
                        phase_bs.append(phase_b())
                    else:
                        phase_bs.append(phase_b())
                        for _ in phase_bs.pop(0):
                            pass
            while phase_bs:
                for _ in phase_bs.pop(0):
                    pass

    if split_waits:
        _split_waits(nc)
    return nc


_NC_CACHE = {}


def _get_nc(nrows):
    if nrows not in _NC_CACHE:
        _NC_CACHE[nrows] = build(nrows)
    return _NC_CACHE[nrows]


def make_aux(H):
    H32 = np.asarray(H, dtype=np.float32)
    hq = (np.sign(H32) * 0.125).astype(np.float16)
    hs = np.sign(H32).astype(np.float16)
    ident = np.eye(P, dtype=np.float16)
    onesc = np.ones((P, 1), dtype=np.float16)
    one1 = np.ones((1, 1), dtype=np.float32)
    return hq, hs, ident, onesc, one1


def block_transpose(xf16):
    """[rows, 4096] fp16 -> [NB, 128, rows] fp16 (pure layout)."""
    r = xf16.shape[0]
    return np.ascontiguousarray(
        xf16.reshape(r, NB, P).transpose(1, 2, 0)
    )


def kernel(x, H):
    x = np.asarray(x)
    orig_shape = x.shape
    xf = x.reshape(-1, NCOLS).astype(np.float16)
    nrows_total = xf.shape[0]
    assert nrows_total % N_CORES == 0
    shard = nrows_total // N_CORES

    hq, hs, ident, onesc, one1 = make_aux(H)
    nc = _get_nc(shard)

    in_maps = [
        {
            "x": block_transpose(xf[i * shard : (i + 1) * shard]),
            "hq": hq,
            "hs": hs,
            "ident": ident,
            "onesc": onesc,
            "one1": one1,
        }
        for i in range(N_CORES)
    ]
    res = run_bass_kernel_spmd(nc, in_maps, core_ids=list(range(N_CORES)))
    out = np.concatenate([r["o"] for r in res.results], axis=0)
    return out.reshape(orig_shape).astype(np.float32)


if __name__ == "__main__":
    rng = np.random.default_rng(0)
    nrows = 512
    x32 = rng.standard_normal((nrows, NCOLS), dtype=np.float32)

    Hnp = np.ones((1, 1))
    while Hnp.shape[0] < P:
        Hnp = np.block([[Hnp, Hnp], [Hnp, -Hnp]])
    Hnp = (Hnp / math.sqrt(P)).astype(np.float32)

    def ref(x, H):
        xr = (x.reshape(-1, NB, P) @ H).reshape(-1, NCOLS)
        std = np.maximum(np.sqrt((xr * xr).mean(-1, keepdims=True)), 1e-8)
        step = ALPHA * std / QMAX
        q = np.clip(np.round(xr / step), -QMAX, QMAX) * step
        return (q.reshape(-1, NB, P) @ H).reshape(-1, NCOLS)

    from concourse.bass_interp import CoreSim
    from concourse.timeline_sim import TimelineSim

    nc = build(nrows, split_waits=False)
    hq, hs, ident, onesc, one1 = make_aux(Hnp)
    x16 = x32.astype(np.float16)
    sim = CoreSim(nc)
    sim.tensor("x")[:] = block_transpose(x16)
    sim.tensor("hq")[:] = hq
    sim.tensor("hs")[:] = hs
    sim.tensor("ident")[:] = ident
    sim.tensor("onesc")[:] = onesc
    sim.tensor("one1")[:] = one1
    sim.simulate()
    got = np.asarray(sim.tensor("o")).astype(np.float32)
    want = ref(x32, Hnp)
    err = np.abs(got - want)
    denom = np.abs(want).max()
    l2 = np.linalg.norm((got - want).ravel()) / np.linalg.norm(want.ravel())
    print("max abs err:", err.max(), "rel l2:", l2)
    bad = (err > 1e-3 * denom).sum()
    print("elements off by >1e-3*scale:", bad, "/", err.size)

    nc2 = build(nrows)
    ts = TimelineSim(nc2)
    ts.simulate()
    print(f"timeline {nrows} rows: {ts.time:.0f}ns")


# revision 8
# speedup vs baseline: 1.1125x; 1.0220x over previous
"""HadamardTrustQuantizer Trainium2 kernel, v4 (host block-transposed input).

Forward math (mask term cancels):
    y   = blockwise_rot(x, H)          # H: 128x128 Hadamard
    std = max(sqrt(mean(y^2, -1)), 1e-8) = max(sqrt(mean(x^2, -1)), 1e-8)
    step = ALPHA*std/QMAX
    q   = clip(round(y/step), -QMAX, QMAX)
    out = blockwise_rot(q*step, H)

v4 strategy (per core, data-parallel shard of 2048 rows):
  - host hands x BLOCK-TRANSPOSED: x_fm[b, f, r] (pure layout change), fp16.
    This kills the on-device x-transposes and their PSUM drains.
  - row sum-of-squares: square chunks on DVE (2x), reduce over features with
    PE ones-matmuls accumulating into a [1, rows] PSUM strip; a K=1 fp32
    matmul transposes the strip to [rows, 1] for per-partition use.
  - mm1 row-major: lhsT = x-strip (stationary), rhs = Hq = sign(H)/8 moving;
    output y'' lands [row, feat] so the ACT drain fuses prescale + round:
    q' = fp16(y''*rs + 1536) = 1536 + round_even(u), rs per-partition.
  - un-offset + top clip fused into the q-transpose drain (DVE 2x):
    (q' - 1536) then min 7; bottom clip (max -7) one-op pass on GPSIMD.
  - mm2: stationary q (feature-major), moving hs = +-1 sign matrix; output
    row-major; ACT/DVE drain applies os = step/sqrt(128) per-partition.
"""

import math
import sys

sys.path.insert(0, "/opt/trn_rl_repo")

import ml_dtypes  # noqa: F401
import numpy as np

import concourse.bass as bass
import concourse.tile as tile
from concourse import mybir
from concourse.bass_utils import run_bass_kernel_spmd

P = 128
NCOLS = 4096
NB = NCOLS // P          # 32 blocks per row
ALPHA = 2.5139
QMAX = 7.0
OFF = 1536.0             # fp16 round-to-integer offset (ulp=1 in [1024,2048))
# s1 = sqrt(ssq * CS) = step*sqrt(2), where step = ALPHA*sqrt(ssq/4096)/QMAX
CS = (ALPHA * math.sqrt(2.0) / (QMAX * 64.0)) ** 2
# os = step/sqrt(128) = s1/16

N_CORES = 8
ROWS_PER_CORE = 2048
RCHUNK = 256             # rows per DMA chunk

F32 = mybir.dt.float32
F16 = mybir.dt.float16
Alu = mybir.AluOpType
Act = mybir.ActivationFunctionType


def _split_waits(nc, maxw_default=1, drain_maxw=1):
    """walrus in this container rejects >1 sem wait per instruction.
    Hoist excess waits onto preceding same-engine NoOps."""
    for bb in nc.m.functions[0].blocks:
        new_list, changed = [], False
        for inst in bb.instructions:
            si = inst.sync_info
            maxw = drain_maxw if type(inst).__name__ == "InstDrain" else maxw_default
            if si is not None and len(si.on_wait) > maxw:
                waits = list(si.on_wait)
                head, tail = waits[:-maxw], waits[-maxw:]
                k = 0
                while head:
                    chunk, head = head[:1], head[1:]
                    nop = mybir.InstNoOp(name=f"{inst.name}-ws{k}", ins=[], outs=[])
                    nop.engine = inst.engine
                    nop.sync_info = mybir.SyncInfo(on_wait=chunk, on_update=[])
                    new_list.append(nop)
                    k += 1
                inst.sync_info = mybir.SyncInfo(
                    on_wait=tail, on_update=list(si.on_update)
                )
                changed = True
            new_list.append(inst)
        if changed:
            bb.instructions = new_list


def build(nrows=ROWS_PER_CORE, split_waits=True,
          act_outdrains=(0, 1, 0, 1, 0, 1, 0, 1),
          alt_outdrains=(0, 1, 0, 1, 1, 1, 0, 1), braid=True):
    """Per-core program; input x_fm[NB, 128, nrows] fp16 (block-transposed)."""
    assert nrows % RCHUNK == 0
    n_chunks = nrows // RCHUNK
    tiles_per_chunk = RCHUNK // P
    n_tiles = nrows // P

    nc = bass.Bass("TRN2", target_bir_lowering=False)
    x_d = nc.dram_tensor("x", [NB, P, nrows], F16, kind="ExternalInput")
    hq_d = nc.dram_tensor("hq", [P, P], F16, kind="ExternalInput")
    hs_d = nc.dram_tensor("hs", [P, P], F16, kind="ExternalInput")
    id_d = nc.dram_tensor("ident", [P, P], F16, kind="ExternalInput")
    ones_d = nc.dram_tensor("onesc", [P, 1], F16, kind="ExternalInput")
    o_d = nc.dram_tensor("o", [nrows, NCOLS], F16, kind="ExternalOutput")

    with tile.TileContext(nc) as tc:
        import contextlib

        with contextlib.ExitStack() as ctx:
            singles = ctx.enter_context(tc.tile_pool(name="singles", bufs=1))
            px = ctx.enter_context(tc.tile_pool(name="px", bufs=3))
            psq = ctx.enter_context(tc.tile_pool(name="psq", bufs=2))
            pq = ctx.enter_context(tc.tile_pool(name="pq", bufs=3))
            pqT = ctx.enter_context(tc.tile_pool(name="pqT", bufs=6))
            pqc = ctx.enter_context(tc.tile_pool(name="pqc", bufs=6))
            pout = ctx.enter_context(tc.tile_pool(name="pout", bufs=3))
            pst = ctx.enter_context(tc.tile_pool(name="pst", bufs=6))
            psT = ctx.enter_context(tc.tile_pool(name="psT", bufs=1, space="PSUM"))
            pyp = ctx.enter_context(tc.tile_pool(name="pyp", bufs=3, space="PSUM"))
            ptq = ctx.enter_context(tc.tile_pool(name="ptq", bufs=2, space="PSUM"))
            pop = ctx.enter_context(tc.tile_pool(name="pop", bufs=2, space="PSUM"))

            x_chunks = {}

            def emit_dma(c):
                if c >= n_chunks:
                    return
                xs = px.tile([P, NB, RCHUNK], F16, tag="x")
                # 4 slab-aligned sub-DMAs so squares can start on slab 0
                # while the rest of the chunk is still in flight
                for s in range(4):
                    b0, b1 = s * (NB // 4), (s + 1) * (NB // 4)
                    srcp = x_d[b0:b1, :, c * RCHUNK : (c + 1) * RCHUNK]
                    nc.sync.dma_start(
                        out=xs[:, b0:b1, :],
                        in_=srcp.rearrange("b p r -> p b r"),
                    )
                x_chunks[c] = xs

            emit_dma(0)

            hq_sb = singles.tile([P, P], F16)
            hs_sb = singles.tile([P, P], F16)
            id_sb = singles.tile([P, P], F16)
            ones_sb = singles.tile([P, 1], F16)
            nc.sync.dma_start(out=hq_sb, in_=hq_d[:])
            nc.sync.dma_start(out=hs_sb, in_=hs_d[:])
            nc.sync.dma_start(out=id_sb, in_=id_d[:])
            nc.sync.dma_start(out=ones_sb, in_=ones_d[:])

            phase_bs = []

            for c in range(n_chunks):
                emit_dma(c + 1)
                xs = x_chunks.pop(c)

                # squares for the whole chunk (DVE 2x), in 4 wide slabs
                xsq = psq.tile([P, NB, RCHUNK], F16, tag="xsq")
                for s in range(4):
                    b0 = s * (NB // 4)
                    b1 = (s + 1) * (NB // 4)
                    nc.vector.tensor_tensor(
                        out=xsq[:, b0:b1, :], in0=xs[:, b0:b1, :],
                        in1=xs[:, b0:b1, :], op=Alu.mult,
                    )

                # stats for all tiles of the chunk, hoisted so rs/os are
                # ready well before the ACT rounds need them
                stats = []
                for t in range(tiles_per_chunk):
                    cs0 = t * P
                    # ssq[rows] directly as [128, 1]: stationary = xsq block,
                    # moving = ones column (N=1, nearly free on PE)
                    sqT = psT.tile([P, 1], F32, tag="sqT")
                    for b in range(NB):
                        nc.tensor.matmul(
                            sqT, lhsT=xsq[:, b, cs0 : cs0 + P],
                            rhs=ones_sb,
                            start=(b == 0), stop=(b == NB - 1),
                        )
                    # rs = 1/(step*sqrt(2)) = sqrt(1/(ssq*CS));
                    # os = step/sqrt(128) = sqrt(ssq*CS)/16
                    rcp = pst.tile([P, 1], F32, tag="rcp")
                    nc.vector.reciprocal(out=rcp, in_=sqT)
                    rs = pst.tile([P, 1], F32, tag="rs")
                    nc.scalar.activation(out=rs, in_=rcp, func=Act.Sqrt,
                                         scale=1.0 / CS)
                    os_t = pst.tile([P, 1], F32, tag="os")
                    nc.scalar.activation(out=os_t, in_=sqT, func=Act.Sqrt,
                                         scale=CS / 256.0)
                    stats.append((rs, os_t))

                for t in range(tiles_per_chunk):
                    rt = c * tiles_per_chunk + t
                    r0 = rt * P
                    cs0 = t * P
                    rs, os_t = stats[t]
                    od_mask = act_outdrains if rt % 2 == 0 else alt_outdrains

                    # mm1 row-major + fused ACT prescale/round/offset
                    q = pq.tile([P, NCOLS], F16, tag="q")
                    for g in range(8):
                        yp = pyp.tile([P, 512], F32, tag="yp")
                        for j in range(4):
                            b = 4 * g + j
                            nc.tensor.matmul(
                                yp[:, j * P : (j + 1) * P],
                                lhsT=xs[:, b, cs0 : cs0 + P],
                                rhs=hq_sb,
                                start=True, stop=True,
                            )
                        nc.scalar.activation(
                            out=q[:, g * 512 : (g + 1) * 512], in_=yp,
                            func=Act.Copy, scale=rs[:, 0:1], bias=OFF,
                        )
                        if braid and phase_bs and g in (2, 7):
                            next(phase_bs[0], None)

                    def phase_b(q=q, os_t=os_t, r0=r0):
                        out_t = pout.tile([P, NCOLS], F16, tag="out")
                        for g in range(4):
                            tq = ptq.tile([P, 1024], F16, tag="tq")
                            for k in range(8):
                                b = 8 * g + k
                                nc.tensor.transpose(
                                    tq[:, k * P : (k + 1) * P],
                                    q[:, b * P : (b + 1) * P],
                                    id_sb,
                                )
                            qT = pqT.tile([P, 1024], F16, tag="qT")
                            nc.vector.tensor_scalar(
                                out=qT, in0=tq, scalar1=OFF, scalar2=QMAX,
                                op0=Alu.subtract, op1=Alu.min,
                            )
                            qc = pqc.tile([P, 1024], F16, tag="qc")
                            for hh in range(2):
                                nc.gpsimd.tensor_scalar(
                                    out=qc[:, hh * 512 : (hh + 1) * 512],
                                    in0=qT[:, hh * 512 : (hh + 1) * 512],
                                    scalar1=-QMAX, scalar2=None,
                                    op0=Alu.max,
                                )
                            for h in range(2):
                                bank = 2 * g + h
                                op_t = pop.tile([P, 512], F32, tag="op")
                                for j in range(4):
                                    k = h * 4 + j
                                    nc.tensor.matmul(
                                        op_t[:, j * P : (j + 1) * P],
                                        lhsT=qc[:, k * P : (k + 1) * P],
                                        rhs=hs_sb,
                                        start=True, stop=True,
                                    )
                                c0 = g * 1024 + h * 512
                                if od_mask[bank]:
                                    nc.scalar.activation(
                                        out=out_t[:, c0 : c0 + 512], in_=op_t,
                                        func=Act.Copy, scale=os_t[:, 0:1],
                                    )
                                else:
                                    nc.vector.tensor_scalar(
                                        out=out_t[:, c0 : c0 + 512], in0=op_t,
                                        scalar1=os_t[:, 0:1], scalar2=None,
                                        op0=Alu.mult,
                                    )
                            if g == 1:
                                nc.sync.dma_start(
                                    out=o_d[r0 : r0 + P, 0:2048],
                                    in_=out_t[:, 0:2048],
                                )
                            elif g == 3:
                                nc.sync.dma_start(
                                    out=o_d[r0 : r0 + P, 2048:4096],
                                    in_=out_t[:, 2048:4096],
                                )
                            yield None
                        yield None

                    if braid:
                        if phase_bs:
                            for _ in phase_bs.pop(0):
                                pass
                        phase_bs.append(phase_b())
                    else:
                        phase_bs.append(phase_b())
                        for _ in phase_bs.pop(0):
                            pass
            while phase_bs:
                for _ in phase_bs.pop(0):
                    pass

    if split_waits:
        _split_waits(nc)
    return nc


_NC_CACHE = {}


def _get_nc(nrows):
    if nrows not in _NC_CACHE:
        _NC_CACHE[nrows] = build(nrows)
    return _NC_CACHE[nrows]


def make_aux(H):
    H32 = np.asarray(H, dtype=np.float32)
    hq = (np.sign(H32) * 0.125).astype(np.float16)
    hs = np.sign(H32).astype(np.float16)
    ident = np.eye(P, dtype=np.float16)
    onesc = np.ones((P, 1), dtype=np.float16)
    return hq, hs, ident, onesc


def block_transpose(xf16):
    """[rows, 4096] fp16 -> [NB, 128, rows] fp16 (pure layout)."""
    r = xf16.shape[0]
    return np.ascontiguousarray(
        xf16.reshape(r, NB, P).transpose(1, 2, 0)
    )


def kernel(x, H):
    x = np.asarray(x)
    orig_shape = x.shape
    xf = x.reshape(-1, NCOLS).astype(np.float16)
    nrows_total = xf.shape[0]
    assert nrows_total % N_CORES == 0
    shard = nrows_total // N_CORES

    hq, hs, ident, onesc = make_aux(H)
    nc = _get_nc(shard)

    in_maps = [
        {
            "x": block_transpose(xf[i * shard : (i + 1) * shard]),
            "hq": hq,
            "hs": hs,
            "ident": ident,
            "onesc": onesc,
        }
        for i in range(N_CORES)
    ]
    res = run_bass_kernel_spmd(nc, in_maps, core_ids=list(range(N_CORES)))
    out = np.concatenate([r["o"] for r in res.results], axis=0)
    return out.reshape(orig_shape).astype(np.float32)


if __name__ == "__main__":
    rng = np.random.default_rng(0)
    nrows = 512
    x32 = rng.standard_normal((nrows, NCOLS), dtype=np.float32)

    Hnp = np.ones((1, 1))
    while Hnp.shape[0] < P:
        Hnp = np.block([[Hnp, Hnp], [Hnp, -Hnp]])
    Hnp = (Hnp / math.sqrt(P)).astype(np.float32)

    def ref(x, H):
        xr = (x.reshape(-1, NB, P) @ H).reshape(-1, NCOLS)
        std = np.maximum(np.sqrt((xr * xr).mean(-1, keepdims=True)), 1e-8)
        step = ALPHA * std / QMAX
        q = np.clip(np.round(xr / step), -QMAX, QMAX) * step
        return (q.reshape(-1, NB, P) @ H).reshape(-1, NCOLS)

    from concourse.bass_interp import CoreSim
    from concourse.timeline_sim import TimelineSim

    nc = build(nrows, split_waits=False)
    hq, hs, ident, onesc = make_aux(Hnp)
    x16 = x32.astype(np.float16)
    sim = CoreSim(nc)
    sim.tensor("x")[:] = block_transpose(x16)
    sim.tensor("hq")[:] = hq
    sim.tensor("hs")[:] = hs
    sim.tensor("ident")[:] = ident
    sim.tensor("onesc")[:] = onesc
    sim.simulate()
    got = np.asarray(sim.tensor("o")).astype(np.float32)
    want = ref(x32, Hnp)
    err = np.abs(got - want)
    denom = np.abs(want).max()
    l2 = np.linalg.norm((got - want).ravel()) / np.linalg.norm(want.ravel())
    print("max abs err:", err.max(), "rel l2:", l2)
    bad = (err > 1e-3 * denom).sum()
    print("elements off by >1e-3*scale:", bad, "/", err.size)

    nc2 = build(nrows)
    ts = TimelineSim(nc2)
    ts.simulate()
    print(f"timeline {nrows} rows: {ts.time:.0f}ns")
